# revision 1
# baseline (speedup 1.0000x reference)
"""Trainium2 Bass kernel for nn_CFAggregator (GNN message passing).

Strategy (B-sharded data parallel over 8 cores, no collectives):
  - Host: pure index preprocessing. Per core, build dedup'd edge weights
    (mask .set() semantics + 1/cnt normalization), split edges into two
    signed-int16 index buckets (table rows 0..65535 rel. base 32768 and
    65536..99999 rel. base 98304), sort each bucket by destination column
    (b_local*4+layer), and build the block-sparse one-hot matrices A that
    map gather slots -> destination columns.
  - Device: dma_gather (custom SWDGE ucode, parallel Q7 queues) fetches the
    per-edge feature rows; PE matmuls G_chunk^T @ A_chunk accumulate the
    normalized neighbor sums directly in PSUM (feature-major layout).
    Everything downstream (Wv/Wk/Wq matmuls, persona softmax, highway
    attention, ELU) runs feature-major so no transposes are needed except
    for the two gathered self-feature tiles.
All feature-table traffic happens on-device; the host only touches index
tensors and small weights.
"""

import numpy as np

import concourse.bass as bass
import concourse.bacc as bacc
import concourse.tile as tile
from concourse import mybir
from concourse.bass_utils import run_bass_kernel_spmd
from concourse.masks import make_identity

F32 = mybir.dt.float32
I32 = mybir.dt.int32
I16 = mybir.dt.int16
AF = mybir.ActivationFunctionType
OP = mybir.AluOpType

# problem dims (hardcoded per contract)
B, MC, U, N, DIN, DOUT, E = 2048, 4, 20000, 100000, 128, 128, 65536
RES_RATE = 0.9
NCORES = 8
BC = B // NCORES          # 256 nodes per core
DEST = BC * MC            # 1024 destination columns per core
P = 128

# int16 bucket bases: bucket0 covers rows [0, 65536), bucket1 covers [65536, 100000)
# (base1 centered so ~50% of idx are non-negative: the gather ucode pops
# trailing negative indices, and each gather piece must end on a >=0 one)
BUCKET_BASES = (32768, 82768)
BUCKET_LO = (0, 65536)
PIECE = 8  # chunks per dma_gather (1024 idx = hard ucode packet limit)

# consts tile slots (each [128, 128]): 4 DMA'd weights + 5 synthesized
(S_WK, S_WQ, S_WVA, S_WVF) = range(4)
(S_ONES, S_ONESC, S_MULO, S_MUHI, S_ID) = range(5)
NSLOT = 4


# --------------------------------------------------------------------------
# host-side preprocessing (index math only)
# --------------------------------------------------------------------------

def _wrap_idx16(idx_flat):
    """int16 index list -> [128, ceil(n/16)] wrapped in 16 partitions, x8 replicas."""
    n = len(idx_flat)
    cols = (n + 15) // 16
    pad = np.zeros(cols * 16, np.int16)
    pad[:n] = idx_flat.astype(np.int16)
    w16 = pad.reshape(cols, 16).T
    return np.ascontiguousarray(np.tile(w16, (8, 1)))


def preprocess(inputs):
    """Build per-core gather/index/one-hot structures. Returns (plan, percore).

    plan: compile-time structure shared by all cores:
        nchk[r]        chunks per bucket r
        segs           list of (bucket, chunk, ps_tile, lo, hi, acol)
        aw             total A columns
    percore: list of dicts with A data, wrapped idx, self idx.
    """
    nodes = np.asarray(inputs["nodes"]).astype(np.int64)
    unique_ids = np.asarray(inputs["unique_ids"]).astype(np.int64)
    row_idx = np.asarray(inputs["row_idx"]).astype(np.int64)
    layer_idx = np.asarray(inputs["layer_idx"]).astype(np.int64)
    col_idx = np.asarray(inputs["col_idx"]).astype(np.int64)

    eff = unique_ids[col_idx]                       # table row per edge
    # dedup (b, layer, col) triples: .set() counts duplicates once
    key = (row_idx * MC + layer_idx) * U + col_idx
    uniq_keys, first_pos = np.unique(key, return_index=True)
    keep = np.zeros(E, bool)
    keep[first_pos] = True
    grp_of_uniq = uniq_keys // U                    # (b*MC+m) per distinct triple
    cnt = np.bincount(grp_of_uniq, minlength=B * MC)
    grp = row_idx * MC + layer_idx
    w = np.where(keep, 1.0 / np.maximum(cnt[grp], 1), 0.0).astype(np.float32)
    dest_all = (row_idx % BC) * MC + layer_idx      # core-local dest column

    # per (core, bucket): dest-sorted edge lists
    core_lists = []   # [core][bucket] -> (idx_rel int32, dest int32, w f32)
    for c in range(NCORES):
        sel = (row_idx >= c * BC) & (row_idx < (c + 1) * BC)
        e_eff, e_dest, e_w = eff[sel], dest_all[sel], w[sel]
        per_bucket = []
        for r in range(2):
            bsel = (e_eff >= BUCKET_LO[r]) & (e_eff < (BUCKET_LO[1] if r == 0 else N))
            order = np.argsort(e_dest[bsel], kind="stable")
            per_bucket.append((
                (e_eff[bsel][order] - BUCKET_BASES[r]).astype(np.int32),
                e_dest[bsel][order].astype(np.int32),
                e_w[bsel][order],
            ))
        core_lists.append(per_bucket)

    # dest-range chunking: chunk k covers a fixed dest window [d0, d1) chosen
    # greedily so every core's edge count in the window fits 128 slots. This
    # makes the per-chunk matmul span exact (no union-over-cores inflation).
    cnt_rcd = np.zeros((2, NCORES, DEST), np.int32)
    for c in range(NCORES):
        for r in range(2):
            d_arr = core_lists[c][r][1]
            if len(d_arr):
                cnt_rcd[r, c] = np.bincount(d_arr, minlength=DEST)
    cuts = []
    nchk = []
    for r in range(2):
        bcuts = []
        d = 0
        while d < DEST:
            tot = np.zeros(NCORES, np.int64)
            d0 = d
            while d < DEST:
                nxt = tot + cnt_rcd[r, :, d]
                if nxt.max() > 128:
                    break
                tot = nxt
                d += 1
            assert d > d0
            bcuts.append((d0, d))
        cuts.append(bcuts)
        nchk.append(len(bcuts))

    # re-layout each core's bucket stream into the dest-window chunks, padding
    # every chunk to 128 slots (pad idx 0 = bucket base row, weight 0)
    core_streams = []   # [core][bucket] -> (idx int32[nchk*128], dest, w)
    for c in range(NCORES):
        per_bucket = []
        for r in range(2):
            idx_rel, dests, ws = core_lists[c][r]
            cap = nchk[r] * 128
            s_idx = np.zeros(cap, np.int32)
            s_dst = np.full(cap, -1, np.int32)
            s_w = np.zeros(cap, np.float32)
            for k, (d0, d1) in enumerate(cuts[r]):
                m = (dests >= d0) & (dests < d1)
                n = int(m.sum())
                assert n <= 128
                s_idx[k * 128:k * 128 + n] = idx_rel[m]
                s_dst[k * 128:k * 128 + n] = dests[m]
                s_w[k * 128:k * 128 + n] = ws[m]
            per_bucket.append((s_idx, s_dst, s_w))
        core_streams.append(per_bucket)

    # gather pieces: big pieces early (amortize SWDGE fixed cost), small tail
    # pieces so late aggregation work trails the transfers closely. Every
    # piece must END on idx >= 0 (the ucode pops trailing negatives).
    pieces = []
    for r in range(2):
        bounds = [0]
        n = nchk[r]
        while n - bounds[-1] > PIECE:
            bounds.append(bounds[-1] + PIECE)
        rem = n - bounds[-1]
        if rem > 8:
            bounds.append(bounds[-1] + (rem + 1) // 2)
        bounds.append(n)
        pieces.append([(bounds[i], bounds[i + 1]) for i in range(len(bounds) - 1)])
    for c in range(NCORES):
        for r in range(2):
            s_idx, s_dst, s_w = core_streams[c][r]
            for (k0, k1) in pieces[r]:
                last = k1 * 128 - 1
                if s_idx[last] >= 0:
                    continue  # pads are idx 0, so this is a real negative edge
                ch0 = (k1 - 1) * 128
                cand = np.nonzero(s_idx[ch0:last] >= 0)[0]
                assert len(cand), "whole final chunk of a piece is negative"
                j = ch0 + cand[0]
                for arr in (s_idx, s_dst, s_w):
                    arr[j], arr[last] = arr[last], arr[j]
    # per-(bucket, chunk) dest span = the fixed window
    spans = []
    for r in range(2):
        for k, (d0, d1) in enumerate(cuts[r]):
            spans.append((r, k, d0, d1))

    # segments: split spans at the 512-column PSUM-tile boundary
    segs = []
    acol = 0
    for (r, k, lo, hi) in spans:
        if lo < 0:
            continue
        for t in range(2):
            b0, b1 = t * 512, (t + 1) * 512
            s0, s1 = max(lo, b0), min(hi, b1)
            if s1 > s0:
                segs.append(dict(bucket=r, chunk=k, tile=t, lo=s0, hi=s1,
                                 acol=acol + (s0 - lo)))
        acol += hi - lo
    aw = max(acol, 1)

    plan = dict(nchk=tuple(nchk), segs=segs, aw=aw,
                pieces=(tuple(pieces[0]), tuple(pieces[1])))

    percore = []
    for c in range(NCORES):
        amat = np.zeros((P, aw), np.float32)
        widx = []
        # per-span A fill: find each span's acol base
        span_acol = {}
        ac = 0
        for (r, k, lo, hi) in spans:
            if lo < 0:
                span_acol[(r, k)] = (ac, lo)
                continue
            span_acol[(r, k)] = (ac, lo)
            ac += hi - lo
        for r in range(2):
            s_idx, s_dst, s_w = core_streams[c][r]
            assert s_idx.max(initial=0) <= 32767 and s_idx.min(initial=0) >= -32768
            widx.append(_wrap_idx16(s_idx))
            for k in range(nchk[r]):
                a0, lo = span_acol[(r, k)]
                sl = slice(k * 128, (k + 1) * 128)
                real = s_dst[sl] >= 0
                pp = np.nonzero(real)[0]
                amat[pp, a0 + s_dst[sl][pp] - lo] = s_w[sl][pp]
        sidx = np.zeros((P, 2), np.int32)
        sidx[:, 0] = nodes[c * BC: c * BC + 128]
        sidx[:, 1] = nodes[c * BC + 128: (c + 1) * BC]
        percore.append(dict(amat=amat, widx0=widx[0], widx1=widx[1], sidx=sidx))

    return plan, percore


def make_consts(inputs):
    """([128, 4*128] weights, [128, 2] mu) shared across cores."""
    c = np.zeros((P, NSLOT * 128), np.float32)
    c[:, S_WK * 128:(S_WK + 1) * 128] = np.asarray(inputs["Wk"], np.float32)
    c[:, S_WQ * 128:(S_WQ + 1) * 128] = np.asarray(inputs["Wq"], np.float32)
    c[:, S_WVA * 128:(S_WVA + 1) * 128] = np.asarray(inputs["Wv_agg"], np.float32)
    c[:, S_WVF * 128:(S_WVF + 1) * 128] = np.asarray(inputs["Wv_ff"], np.float32)
    mu = np.asarray(inputs["mu_w"]).astype(np.float32).reshape(2, DOUT).T.copy()
    return c, mu


# --------------------------------------------------------------------------
# device module
# --------------------------------------------------------------------------

def build_module(plan):
    nchk = plan["nchk"]
    aw = plan["aw"]
    # PE program order: tile-0 segs first so half-0 stats can start while
    # tile-1 aggregation is still consuming late gather pieces.
    segs = sorted(plan["segs"], key=lambda s: (s["tile"], s["bucket"], s["chunk"]))

    nc = bacc.Bacc("TRN2", target_bir_lowering=False, debug=False,
                   num_devices=NCORES, num_swdge_queues=4)

    agg_t = nc.dram_tensor("agg_table", [N, DIN], F32, kind="ExternalInput")
    ff_t = nc.dram_tensor("ff_table", [N, DIN], F32, kind="ExternalInput")
    consts = nc.dram_tensor("consts", [P, NSLOT * 128], F32, kind="ExternalInput")
    mu_d = nc.dram_tensor("mu", [P, 2], F32, kind="ExternalInput")
    amat = nc.dram_tensor("amat", [P, aw], F32, kind="ExternalInput")
    widx0 = nc.dram_tensor("widx0", [P, nchk[0] * 8], I16, kind="ExternalInput")
    widx1 = nc.dram_tensor("widx1", [P, nchk[1] * 8], I16, kind="ExternalInput")
    sidx = nc.dram_tensor("sidx", [P, 2], I32, kind="ExternalInput")
    out_t = nc.dram_tensor("out", [2, P, BC], F32, kind="ExternalOutput")

    with tile.TileContext(nc) as tc:
        with (
            tc.tile_pool(name="sb", bufs=1) as sb,
            tc.tile_pool(name="psA", bufs=4, space="PSUM") as psA,
            tc.tile_pool(name="ps", bufs=2, space="PSUM") as ps,
        ):
            def slot(k):
                return c_sb[:, k * 128:(k + 1) * 128]

            def syn(k):
                return syn_sb[:, k * 128:(k + 1) * 128]

            # ---- input DMAs (tiny index tensors first; A-matrix on the ACT
            # HWDGE ring so it never blocks the gather-index loads)
            c_sb = sb.tile([P, NSLOT * 128], F32, tag="c_sb")
            a_sb = sb.tile([P, aw], F32, tag="a_sb")
            mu_sb = sb.tile([P, 2], F32, tag="mu_sb")
            w0_sb = sb.tile([P, nchk[0] * 8], I16, tag="w0_sb")
            w1_sb = sb.tile([P, nchk[1] * 8], I16, tag="w1_sb")
            si_sb = sb.tile([P, 2], I32, tag="si_sb")
            nc.sync.dma_start(out=w0_sb[:], in_=widx0[:, :])
            nc.sync.dma_start(out=w1_sb[:], in_=widx1[:, :])
            nc.sync.dma_start(out=si_sb[:], in_=sidx[:, :])
            nc.sync.dma_start(out=c_sb[:], in_=consts[:, :])
            nc.sync.dma_start(out=mu_sb[:], in_=mu_d[:, :])
            nc.scalar.dma_start(out=a_sb[:], in_=amat[:, :])
            # prime the ACT function table with the Sqrt set at t=0 (copies and
            # squares live in every set, so the only midstream load left is Exp)
            warm = sb.tile([P, 1], F32, tag="warm")
            nc.vector.memset(warm[:], 1.0)
            warm2 = sb.tile([P, 1], F32, tag="warm2")
            nc.scalar.sqrt(warm2[:], warm[:])
            # synthesized constants: ones, ones/DOUT, mu broadcasts, identity
            syn_sb = sb.tile([P, 5 * 128], F32, tag="syn_sb")
            nc.vector.memset(syn_sb[:, S_ONES * 128:(S_ONES + 1) * 128], 1.0)
            nc.vector.memset(syn_sb[:, S_ONESC * 128:(S_ONESC + 1) * 128], 1.0 / DOUT)
            nc.vector.tensor_copy(syn_sb[:, S_MULO * 128:(S_MULO + 1) * 128],
                                  mu_sb[:, 0:1].to_broadcast((P, 128)))
            nc.vector.tensor_copy(syn_sb[:, S_MUHI * 128:(S_MUHI + 1) * 128],
                                  mu_sb[:, 1:2].to_broadcast((P, 128)))
            make_identity(nc, syn_sb[:, S_ID * 128:(S_ID + 1) * 128])

            # ---- dummy gather: triggers the mlp ucode library load at t=0 so
            # the real gathers don't pay the ~4us IRAM fetch after the idx DMAs
            dum_i = sb.tile([P, 8], I16, tag="dum_i")
            nc.gpsimd.memset(dum_i[:], 0)
            dum_o = sb.tile([P, 1, 64], F32, tag="dum_o")
            nc.gpsimd.dma_gather(dum_o[:], agg_t[:, 0:64], dum_i[:], 128, 128, 64,
                                 elem_step=128, queue_num=1)

            # ---- edge gathers (bucket-interleaved pieces on SWDGE queues 1-3)
            g0 = sb.tile([P, nchk[0], 128], F32, tag="g0")
            g1 = sb.tile([P, nchk[1], 128], F32, tag="g1")
            gq = 0
            tiles_w = ((g0, w0_sb), (g1, w1_sb))
            order = []
            np0, np1 = len(plan["pieces"][0]), len(plan["pieces"][1])
            for i in range(max(np0, np1)):
                if i < np0:
                    order.append((0, plan["pieces"][0][i]))
                if i < np1:
                    order.append((1, plan["pieces"][1][i]))
            for r, (k0, k1) in order:
                gt, wt = tiles_w[r]
                nc.gpsimd.dma_gather(
                    gt[:, k0:k1, :], agg_t[BUCKET_BASES[r]:, :],
                    wt[:, k0 * 8:k1 * 8],
                    (k1 - k0) * 128, (k1 - k0) * 128, 128,
                    queue_num=1 + (gq % 3))
                gq += 1

            # ---- self-feature gathers (queue 0, row-major)
            sr_agg = sb.tile([P, 256], F32, tag="sr_agg")
            sr_ff = sb.tile([P, 256], F32, tag="sr_ff")
            for h in range(2):
                nc.gpsimd.indirect_dma_start(
                    out=sr_agg[:, h * 128:(h + 1) * 128], out_offset=None,
                    in_=agg_t[:, :],
                    in_offset=bass.IndirectOffsetOnAxis(ap=si_sb[:, h:h + 1], axis=0))
                nc.gpsimd.indirect_dma_start(
                    out=sr_ff[:, h * 128:(h + 1) * 128], out_offset=None,
                    in_=ff_t[:, :],
                    in_offset=bass.IndirectOffsetOnAxis(ap=si_sb[:, h:h + 1], axis=0))

            # pair_T: [agg_h0 | ff_h0 | agg_h1 | ff_h1], feature-major
            pair_T = sb.tile([P, 512], F32, tag="pair_T")
            for h in range(2):
                tp = ps.tile([P, 128], F32, tag="ps_rot", name=f"tpa{h}")
                nc.tensor.transpose(tp[:], sr_agg[:, h * 128:(h + 1) * 128], syn(S_ID))
                nc.scalar.copy(pair_T[:, h * 256:h * 256 + 128], tp[:])
                tp2 = ps.tile([P, 128], F32, tag="ps_rot", name=f"tpf{h}")
                nc.tensor.transpose(tp2[:], sr_ff[:, h * 128:(h + 1) * 128], syn(S_ID))
                nc.scalar.copy(pair_T[:, h * 256 + 128:(h + 1) * 256], tp2[:])

            # ---- aggregation psums + per-half dense stage, ordered so that
            # everything for half 0 precedes the tile-1 aggregation segs
            pagg = [psA.tile([P, 512], F32, tag="pagg", name=f"pagg{i}") for i in range(2)]
            nc.vector.memset(pagg[0][:], 0.0)
            nc.vector.memset(pagg[1][:], 0.0)
            gtiles = (g0, g1)
            last_per_tile = {}
            for i, s in enumerate(segs):
                last_per_tile[s["tile"]] = i

            # act: [neighT_h0 512 | neighT_h1 512 | selfT_h0 128 | selfT_h1 128]
            act = sb.tile([P, 1280], F32, tag="act")
            vf = sb.tile([P, 256], F32, tag="vf")
            sq = sb.tile([P, 1280], F32, tag="sq")
            den2 = sb.tile([P, 1024], F32, tag="den2")
            num = sb.tile([P, 1024], F32, tag="num")
            den = sb.tile([P, 1024], F32, tag="den")
            rden = sb.tile([P, 1024], F32, tag="rden")
            logit = sb.tile([P, 1024], F32, tag="logit")
            rep4 = lambda apx: apx[:, :, None].to_broadcast((P, 128, MC))
            sts_sb = sb.tile([P, 256], F32, tag="sts_sb")
            stsm_sb = sb.tile([P, 256], F32, tag="stsm_sb")
            stn, stnm = [], []
            dif_ps = ps.tile([P, 512], F32, tag="ps_dif", name="dif_ps", bufs=1)
            kts, qts = [], []

            for h in range(2):
                for i, s in enumerate(segs):
                    if s["tile"] != h:
                        continue
                    nc.tensor.matmul(
                        out=pagg[h][:, s["lo"] - h * 512: s["hi"] - h * 512],
                        lhsT=gtiles[s["bucket"]][:, s["chunk"], :],
                        rhs=a_sb[:, s["acol"]: s["acol"] + s["hi"] - s["lo"]],
                        start=False, stop=(last_per_tile[h] == i),
                        skip_group_check=True)
                nraw = sb.tile([P, 512], F32, tag=f"nraw{h}", name=f"nraw{h}")
                nc.scalar.copy(nraw[:], pagg[h][:])
                p_self = pair_T[:, h * 256:h * 256 + 128]
                p_ff = pair_T[:, h * 256 + 128:(h + 1) * 256]
                p_pair = pair_T[:, h * 256:(h + 1) * 256]
                kt = sb.tile([P, 256], F32, tag=f"kt{h}", name=f"kt{h}")
                qt = sb.tile([P, 256], F32, tag=f"qt{h}", name=f"qt{h}")
                kts.append(kt)
                qts.append(qt)
                for (sl, rhs, dst, o0, o1) in (
                        (S_WVA, nraw[:], act, h * 512, (h + 1) * 512),
                        (S_WVA, p_self, act, 1024 + h * 128, 1024 + (h + 1) * 128),
                        (S_WVF, p_ff, vf, h * 128, (h + 1) * 128),
                        (S_WK, p_pair, kt, 0, 256),
                        (S_WQ, p_pair, qt, 0, 256)):
                    pt = ps.tile([P, o1 - o0], F32, tag="ps_rot",
                                 name=f"pt{h}_{dst.name}_{o0}")
                    nc.tensor.matmul(out=pt[:], lhsT=slot(sl), rhs=rhs,
                                     start=True, stop=True)
                    nc.scalar.copy(dst[:, o0:o1], pt[:])
                # squares + per-half stats into the shared wide psums
                nc.scalar.square(sq[:, h * 512:(h + 1) * 512],
                                 act[:, h * 512:(h + 1) * 512])
                nc.scalar.square(sq[:, 1024 + h * 128:1024 + (h + 1) * 128],
                                 act[:, 1024 + h * 128:1024 + (h + 1) * 128])
                sts_h = ps.tile([P, 128], F32, tag="ps_rot", name=f"sts{h}")
                nc.tensor.matmul(out=sts_h[:], lhsT=syn(S_ONES),
                                 rhs=sq[:, 1024 + h * 128:1024 + (h + 1) * 128],
                                 start=True, stop=True)
                nc.scalar.copy(sts_sb[:, h * 128:(h + 1) * 128], sts_h[:])
                stsm_h = ps.tile([P, 128], F32, tag="ps_rot", name=f"stsm{h}")
                nc.tensor.matmul(out=stsm_h[:], lhsT=syn(S_MULO),
                                 rhs=act[:, 1024 + h * 128:1024 + (h + 1) * 128],
                                 start=True, stop=True)
                nc.scalar.copy(stsm_sb[:, h * 128:(h + 1) * 128], stsm_h[:])
                stn_h = psA.tile([P, 512], F32, tag="pagg", name=f"stn{h}")
                nc.tensor.matmul(out=stn_h[:], lhsT=syn(S_ONES),
                                 rhs=sq[:, h * 512:(h + 1) * 512], start=True, stop=True)
                stn.append(stn_h)
                stnm_h = psA.tile([P, 512], F32, tag="pagg", name=f"stnm{h}")
                nc.tensor.matmul(out=stnm_h[:], lhsT=syn(S_MUHI),
                                 rhs=act[:, h * 512:(h + 1) * 512], start=True, stop=True)
                stnm.append(stnm_h)
                # highway front: dif_i = colsum(K_i * (Q_agg - Q_ff)) / DOUT
                qd = sb.tile([P, 128], F32, tag=f"qd{h}", name=f"qd{h}")
                nc.vector.tensor_sub(qd[:], qt[:, 0:128], qt[:, 128:256])
                pd = sb.tile([P, 256], F32, tag=f"pd{h}", name=f"pd{h}")
                nc.vector.tensor_mul(pd[:, 0:128], kt[:, 0:128], qd[:])
                nc.vector.tensor_mul(pd[:, 128:256], kt[:, 128:256], qd[:])
                nc.tensor.matmul(out=dif_ps[:, h * 256:(h + 1) * 256],
                                 lhsT=syn(S_ONESC), rhs=pd[:],
                                 start=True, stop=True, skip_group_check=True)

                # per-half norm chain: the h0 part overlaps tile-1 aggregation
                nc.vector.tensor_tensor(
                    out=den2[:, h * 512:(h + 1) * 512].rearrange("p (b m) -> p b m", m=MC),
                    in0=stn_h[:].rearrange("p (b m) -> p b m", m=MC),
                    in1=rep4(sts_sb[:, h * 128:(h + 1) * 128]), op=OP.add)
                nc.vector.tensor_tensor(
                    out=num[:, h * 512:(h + 1) * 512].rearrange("p (b m) -> p b m", m=MC),
                    in0=stnm_h[:].rearrange("p (b m) -> p b m", m=MC),
                    in1=rep4(stsm_sb[:, h * 128:(h + 1) * 128]), op=OP.add)
                den_i = nc.scalar.sqrt(den[:, h * 512:(h + 1) * 512],
                                       den2[:, h * 512:(h + 1) * 512])
                nc.vector.reciprocal(rden[:, h * 512:(h + 1) * 512],
                                     den[:, h * 512:(h + 1) * 512])
                nc.vector.tensor_mul(logit[:, h * 512:(h + 1) * 512],
                                     num[:, h * 512:(h + 1) * 512],
                                     rden[:, h * 512:(h + 1) * 512])

            # early ACT helpers
            self_half = sb.tile([P, 256], F32, tag="self_half")
            nc.scalar.mul(self_half[:], act[:, 1024:1280], 0.5)
            vf01 = sb.tile([P, 256], F32, tag="vf01")
            nc.scalar.mul(vf01[:], vf[:], 1.0 - RES_RATE)

            # ---- persona chain tail (esm onward; per-half norm ran above)
            # highway weights (Exp ops ordered after the Sqrt to avoid
            # bouncing the ACT function table)
            wgt_full = sb.tile([P, 512], F32, tag="wgt_full")  # [waa | wfa]
            for h in range(2):
                eneg = sb.tile([P, 256], F32, tag=f"eneg{h}", name=f"eneg{h}")
                e_i = nc.scalar.activation(eneg[:], dif_ps[:, h * 256:(h + 1) * 256], AF.Exp, scale=-1.0)
                tile.add_dep_helper(e_i.ins, den_i.ins, sync=False, reason="act table order")
                wden = sb.tile([P, 256], F32, tag=f"wden{h}", name=f"wden{h}")
                nc.vector.tensor_scalar_add(wden[:], eneg[:], 1.0)
                nc.vector.reciprocal(
                    wgt_full[:].rearrange("p (k c) -> p k c", c=128)[:, h::2, :],
                    wden[:].rearrange("p (k c) -> p k c", c=128))
            esm = sb.tile([P, 1024], F32, tag="esm")
            nc.scalar.activation(esm[:], logit[:], AF.Exp)
            tmul = sb.tile([P, 1024], F32, tag="tmul")
            nc.vector.tensor_mul(tmul[:], esm[:], act[:, 0:1024])
            tsum = sb.tile([P, 256], F32, tag="tsum")
            nc.vector.reduce_sum(
                out=tsum[:], in_=tmul[:].rearrange("p (b m) -> p b m", m=MC),
                axis=mybir.AxisListType.X)
            ssum = sb.tile([P, 256], F32, tag="ssum")
            nc.vector.reduce_sum(
                out=ssum[:], in_=esm[:].rearrange("p (b m) -> p b m", m=MC),
                axis=mybir.AxisListType.X)
            rsum = sb.tile([P, 256], F32, tag="rsum")
            nc.vector.reciprocal(rsum[:], ssum[:])
            nsum = sb.tile([P, 256], F32, tag="nsum")
            nc.vector.tensor_mul(nsum[:], tsum[:], rsum[:])
            vmid = sb.tile([P, 256], F32, tag="vmid")
            nc.vector.scalar_tensor_tensor(
                out=vmid[:], in0=nsum[:], scalar=0.5, in1=self_half[:],
                op0=OP.mult, op1=OP.add)

            # residual mix:
            #   pre_agg = 0.9*vmid + 0.1*vf + 0.1*waa*dd
            #   pre_ff  = vf + 0.1*wfa*dd          (0.9vf + 0.1(vf + wfa dd))
            dd = sb.tile([P, 256], F32, tag="dd")
            nc.vector.tensor_sub(dd[:], vmid[:], vf[:])
            base = sb.tile([P, 256], F32, tag="base")
            nc.vector.scalar_tensor_tensor(
                out=base[:], in0=vmid[:], scalar=RES_RATE, in1=vf01[:],
                op0=OP.mult, op1=OP.add)
            pre = sb.tile([P, 512], F32, tag="pre")   # [agg 256 | ff 256]
            for o, b9 in enumerate((base, vf)):
                nw = sb.tile([P, 256], F32, tag=f"nw{o}", name=f"nw{o}")
                nc.vector.tensor_mul(nw[:], wgt_full[:, o * 256:(o + 1) * 256], dd[:])
                nc.vector.scalar_tensor_tensor(
                    out=pre[:, o * 256:(o + 1) * 256], in0=nw[:],
                    scalar=1.0 - RES_RATE, in1=b9[:], op0=OP.mult, op1=OP.add)

            # ---- ELU: relu(x) + min(exp(x), 1) - 1  (both on the Exp table)
            out_sb = sb.tile([P, 512], F32, tag="out_sb")
            epre = sb.tile([P, 512], F32, tag="epre")
            nc.scalar.activation(epre[:], pre[:], AF.Exp)
            rpre = sb.tile([P, 512], F32, tag="rpre")
            nc.scalar.activation(rpre[:], pre[:], AF.Relu)
            emin = sb.tile([P, 512], F32, tag="emin")
            nc.vector.tensor_scalar(out=emin[:], in0=epre[:], scalar1=1.0,
                                    scalar2=-1.0, op0=OP.min, op1=OP.add)
            nc.vector.tensor_add(out_sb[:], emin[:], rpre[:])

            nc.sync.dma_start(
                out=out_t[:, :, :].rearrange("c d b -> d c b"), in_=out_sb[:])

    nc.compile()
    return nc


# --------------------------------------------------------------------------
# numpy simulation of the device pipeline (for validating preprocessing)
# --------------------------------------------------------------------------

def numpy_simulate(inputs, plan, percore):
    agg_table = np.asarray(inputs["agg_table"], np.float32)
    ff_table = np.asarray(inputs["ff_table"], np.float32)
    cmat, mu2 = make_consts(inputs)
    outs_a, outs_f = [], []
    for c in range(NCORES):
        pc = percore[c]
        # gathers
        def unwrap(widx, nchunks):
            w16 = widx[:16, :]
            return w16.T.reshape(-1).astype(np.int32)[: nchunks * 128]
        g = []
        for r, widx in enumerate((pc["widx0"], pc["widx1"])):
            idx = unwrap(widx, plan["nchk"][r]) + BUCKET_BASES[r]
            g.append(agg_table[idx].reshape(plan["nchk"][r], 128, 128).transpose(1, 0, 2))
        sr_agg = agg_table[pc["sidx"].T.reshape(-1)]   # [256,128] node-major
        sr_ff = ff_table[pc["sidx"].T.reshape(-1)]
        pair_T = np.concatenate([sr_agg.T, sr_ff.T], axis=1)  # [128, 512]
        # aggregation
        pagg = np.zeros((2, P, 512), np.float32)
        for s in plan["segs"]:
            G = g[s["bucket"]][:, s["chunk"], :]               # [128 slots, 128 d]
            A = pc["amat"][:, s["acol"]: s["acol"] + s["hi"] - s["lo"]]
            pagg[s["tile"]][:, s["lo"] - s["tile"] * 512: s["hi"] - s["tile"] * 512] += G.T @ A
        neigh_rawT = np.concatenate([pagg[0], pagg[1]], axis=1)  # [128, 1024]
        Wva = cmat[:, S_WVA * 128:(S_WVA + 1) * 128]
        Wvf = cmat[:, S_WVF * 128:(S_WVF + 1) * 128]
        Wk = cmat[:, S_WK * 128:(S_WK + 1) * 128]
        Wq = cmat[:, S_WQ * 128:(S_WQ + 1) * 128]
        act = np.concatenate([Wva.T @ neigh_rawT, Wva.T @ pair_T[:, 0:256]], axis=1)
        vf = Wvf.T @ pair_T[:, 256:512]
        kt = Wk.T @ pair_T
        qt = Wq.T @ pair_T
        sq = act * act
        n2 = sq[:, 0:1024].sum(0)
        s2 = sq[:, 1024:1280].sum(0)
        mu_lo = mu2[:, 0:1]
        mu_hi = mu2[:, 1:2]
        nmu = (mu_hi * act[:, 0:1024]).sum(0)
        smu = (mu_lo * act[:, 1024:1280]).sum(0)
        den2 = n2 + np.repeat(s2, MC)
        numv = nmu + np.repeat(smu, MC)
        logit = numv / np.sqrt(den2)
        e = np.exp(logit).reshape(BC, MC)
        coef = e / e.sum(1, keepdims=True)
        neighT = act[:, 0:1024].reshape(P, BC, MC)
        nsum = (neighT * coef[None]).sum(-1)                    # [128, 256]
        vmid = 0.5 * (act[:, 1024:1280] + nsum)
        saa = (kt[:, 0:256] * qt[:, 0:256]).sum(0) / DOUT
        saf = (kt[:, 0:256] * qt[:, 256:512]).sum(0) / DOUT
        sfa = (kt[:, 256:512] * qt[:, 0:256]).sum(0) / DOUT
        sff = (kt[:, 256:512] * qt[:, 256:512]).sum(0) / DOUT
        waa = 1.0 / (1.0 + np.exp(-(saa - saf)))
        wfa = 1.0 / (1.0 + np.exp(-(sfa - sff)))
        dd = vmid - vf
        new0 = vf + waa[None] * dd
        new1 = vf + wfa[None] * dd
        pre0 = RES_RATE * vmid + (1 - RES_RATE) * new0
        pre1 = RES_RATE * vf + (1 - RES_RATE) * new1
        elu = lambda x: np.where(x > 0, x, np.exp(np.minimum(x, 0)) - 1)
        outs_a.append(elu(pre0).T)                              # [256, 128]
        outs_f.append(elu(pre1).T)
    return np.concatenate(outs_a, 0), np.concatenate(outs_f, 0)


# --------------------------------------------------------------------------
# public entry point
# --------------------------------------------------------------------------

_module_cache = {}
_last_results = None  # BassKernelResults of the most recent kernel() call


def _plan_signature(plan):
    return (plan["nchk"], plan["aw"], plan["pieces"],
            tuple((s["bucket"], s["chunk"], s["tile"], s["lo"], s["hi"], s["acol"])
                  for s in plan["segs"]))


def kernel(**inputs):
    plan, percore = preprocess(inputs)
    sig = _plan_signature(plan)
    if sig not in _module_cache:
        _module_cache[sig] = build_module(plan)
    nc = _module_cache[sig]

    cmat, mu2 = make_consts(inputs)
    agg_table = np.ascontiguousarray(np.asarray(inputs["agg_table"], np.float32))
    ff_table = np.ascontiguousarray(np.asarray(inputs["ff_table"], np.float32))
    in_maps = []
    for c in range(NCORES):
        pc = percore[c]
        in_maps.append({
            "agg_table": agg_table,
            "ff_table": ff_table,
            "consts": cmat,
            "mu": mu2,
            "amat": pc["amat"],
            "widx0": pc["widx0"],
            "widx1": pc["widx1"],
            "sidx": pc["sidx"],
        })
    res = run_bass_kernel_spmd(nc, in_maps, core_ids=list(range(NCORES)))
    global _last_results
    _last_results = res
    agg_out = np.concatenate([res.results[c]["out"][0].T for c in range(NCORES)], axis=0)
    ff_out = np.concatenate([res.results[c]["out"][1].T for c in range(NCORES)], axis=0)
    return agg_out, ff_out

